# revision 1
# baseline (speedup 1.0000x reference)
"""Trainium2 (Bass/Tile) 8-core kernel for a dense transformer block.

Math (mirrors the reference):
    q      = x @ wi                       # all heads share wi -> q == k == v
    P      = softmax(mask(q q^T / 32))
    head   = q + P @ q
    h      = head @ W_eff + bias          # cat of identical heads @ out_kernel
                                          # == head @ (sum of the 8 blocks)
    hn     = layernorm(h)                 # E[x^2]-E[x]^2 variance, eps=1e-5
    out    = silu(hn @ wi) @ wi

Sharding (8 NeuronCores, one SPMD NEFF):
    core c -> batch c//4, two 256-row strips {j, 7-j} (j = c%4) of that batch
    (balances causal attention load).  q^T and q (both layouts, bf16) are
    AllGathered within each 4-core batch group; W_eff is computed by an 8-core
    AllReduce of per-core out_kernel blocks.  The program is identical on all
    cores: every per-core difference (which rows, which weight block, the
    attention mask) is carried by input data, incl. a host-built additive
    mask tile so arbitrary masks are supported.
"""

import sys

for _p in ("/opt/trn_rl_repo",):
    if _p not in sys.path:
        sys.path.insert(0, _p)

import numpy as np

B, S, D, H = 2, 2048, 1024, 8
NCORES = 8
TOK = 512          # tokens (q rows) per core
NSTR = 8           # 256-row strips per batch
STR = 256          # strip size
KT = S // 128      # 16 k-tiles of 128
EPS = 1e-5
MASK_NEG = -1.0e6  # pre-scale additive mask value (exp(-1e6/32) == 0)

_CACHE = {}


def _strips(j):
    return (j, NSTR - 1 - j)


def _build(debug=False, reps=1, sim_cc_as_dma=False):
    import concourse.bacc as bacc
    import concourse.mybir as mybir
    import concourse.tile as tile
    from concourse.replica_groups import maybe_share_collective_output_space

    dt = mybir.dt
    BF, F32 = dt.bfloat16, dt.float32
    AF = mybir.ActivationFunctionType
    AX = mybir.AxisListType
    ALU = mybir.AluOpType

    nc = bacc.Bacc("TRN2", target_bir_lowering=False, debug=False,
                   num_devices=NCORES)

    # ---------------- I/O (per-core shapes) ----------------
    xt_d = nc.dram_tensor("xt", [D, TOK], F32, kind="ExternalInput")
    wi_d = nc.dram_tensor("wi", [D, D], F32, kind="ExternalInput")
    wo_d = nc.dram_tensor("wo", [D, D], F32, kind="ExternalInput")
    bias_d = nc.dram_tensor("bias", [1, D], F32, kind="ExternalInput")
    amask_d = nc.dram_tensor("amask", [4, 128, S], BF, kind="ExternalInput")
    out_d = nc.dram_tensor("out", [TOK, D], F32, kind="ExternalOutput")
    if debug:
        dbg = {
            "dbg_qT_my": nc.dram_tensor("dbg_qT_my", [128, 8, TOK], BF, kind="ExternalOutput"),
            "dbg_qT_all": nc.dram_tensor("dbg_qT_all", [128, 8, S], BF, kind="ExternalOutput"),
            "dbg_qn_all": nc.dram_tensor("dbg_qn_all", [128, KT, D], BF, kind="ExternalOutput"),
            "dbg_weff": nc.dram_tensor("dbg_weff", [128, 8, D], BF, kind="ExternalOutput"),
            "dbg_E": nc.dram_tensor("dbg_E", [4, 128, S], BF, kind="ExternalOutput"),
            "dbg_hT": nc.dram_tensor("dbg_hT", [2, 128, 8, STR], BF, kind="ExternalOutput"),
            "dbg_hn": nc.dram_tensor("dbg_hn", [128, 4, D], BF, kind="ExternalOutput"),
        }

    # ---------------- collective buffers -------------------
    AR_G = [list(range(NCORES))]
    AG_G = [[0, 1, 2, 3], [4, 5, 6, 7]]
    wred_in = nc.dram_tensor("wred_in", [D, D], BF)
    wred_out = nc.dram_tensor(
        "wred_out", [D, D], BF,
        addr_space=maybe_share_collective_output_space("AllReduce", AR_G))
    qtg_in = nc.dram_tensor("qtg_in", [D * TOK], BF)       # q^T pack, flat
    qtg_out = nc.dram_tensor(
        "qtg_out", [4 * D * TOK], BF,
        addr_space=maybe_share_collective_output_space("AllGather", AG_G))

    with tile.TileContext(nc) as tc:
        with (
            tc.tile_pool(name="persist", bufs=1) as pp,
            tc.tile_pool(name="load", bufs=4) as loadp,
            tc.tile_pool(name="ps", bufs=6, space="PSUM") as psp,
            tc.tile_pool(name="pv", bufs=2, space="PSUM") as pvp,
            tc.tile_pool(name="E", bufs=3) as ep,
            tc.tile_pool(name="ET", bufs=2) as etp,
            tc.tile_pool(name="hT", bufs=2) as htp,
            tc.tile_pool(name="mk", bufs=2) as mkp,
            tc.tile_pool(name="sq", bufs=1) as sqp,
            tc.tile_pool(name="outb", bufs=3) as outp,
            tc.tile_pool(name="small", bufs=1) as smp,
        ):
            # persistent SBUF tensors
            wi_bf = pp.tile([128, 8, D], BF, tag="wi_bf")
            weff_bf = pp.tile([128, 8, D], BF, tag="weff_bf")
            xt_bf = pp.tile([128, 8, TOK], BF, tag="xt_bf")
            qT_my = pp.tile([128, 8, TOK], BF, tag="qT_my")
            qT_all = pp.tile([128, 8, S], BF, tag="qT_all")
            qn_all = pp.tile([128, KT, D], BF, tag="qn_all")
            hn_sb = pp.tile([128, 4, D], BF, tag="hn_sb")
            saT = pp.tile([128, 8, TOK], BF, tag="saT")

            ones1 = smp.tile([1, 128], BF, tag="ones1")
            bias_bf = smp.tile([1, D], BF, tag="bias_bf")
            acc = smp.tile([128, 16], F32, tag="acc")
            eps_ap = smp.tile([128, 1], F32, tag="eps_ap")
            nc.vector.memset(eps_ap[:], EPS)
            rinv = smp.tile([128, 4], F32, tag="rinv")
            st = smp.tile([128, 16], F32, tag="st")

            nc.vector.memset(ones1[:], 1.0)

            for rep in range(reps):
                # ---------- phase 0: loads + casts (x, wi first: they gate q);
                # the W_eff chain (wo -> bf16 -> AllReduce) follows and overlaps
                # everything up to the out-projection.
                for i in range(4):
                    xf = loadp.tile([128, 2, TOK], F32, tag="ld")
                    nc.sync.dma_start(
                        xf[:], xt_d[256 * i:256 * (i + 1), :]
                        .rearrange("(g p) t -> p g t", p=128))
                    nc.vector.tensor_copy(xt_bf[:, 2 * i:2 * (i + 1), :], xf[:])
                for kt in range(8):
                    wf = loadp.tile([128, D], F32, tag="ld")
                    nc.sync.dma_start(wf[:], wi_d[128 * kt:128 * (kt + 1), :])
                    eng = nc.scalar if kt % 2 == 0 else nc.vector
                    if eng is nc.scalar:
                        eng.copy(wi_bf[:, kt, :], wf[:])
                    else:
                        eng.tensor_copy(wi_bf[:, kt, :], wf[:])

                # ---------- phase 1: q = x @ wi (once); q^T via DMA-transpose;
                # AllGather both layouts (bf16) within the 4-core batch group.
                qn_my = pp.tile([128, 4, D], BF, tag="hnT", name=f"qn_my{rep}")
                for tt in range(4):
                    for hhalf in range(2):
                        qn_ps = psp.tile([128, TOK], F32, tag="ps")
                        for kd in range(8):
                            nc.tensor.matmul(
                                qn_ps[:], xt_bf[:, kd, 128 * tt:128 * (tt + 1)],
                                wi_bf[:, kd, 512 * hhalf:512 * (hhalf + 1)],
                                start=(kd == 0), stop=(kd == 7))
                        nc.scalar.copy(qn_my[:, tt, 512 * hhalf:512 * (hhalf + 1)],
                                       qn_ps[:])
                for tt in range(4):
                    nc.scalar.dma_start_transpose(
                        qT_my[:, :, 128 * tt:128 * (tt + 1)], qn_my[:, tt, :])
                nc.sync.dma_start(
                    qtg_in.ap().rearrange("(m p t) -> p m t", p=128, m=8),
                    qT_my[:])
                if sim_cc_as_dma:
                    for r in range(4):
                        nc.sync.dma_start(
                            qtg_out[r * D * TOK:(r + 1) * D * TOK], qtg_in[:])
                else:
                    nc.gpsimd.collective_compute(
                        "AllGather", ALU.bypass, replica_groups=AG_G,
                        ins=[qtg_in.ap().opt()], outs=[qtg_out.ap().opt()])

                # ---------- W_eff chain (big slack: needed only at out-proj) ----
                for kt in range(8):
                    wof = loadp.tile([128, D], F32, tag="ld")
                    nc.sync.dma_start(wof[:], wo_d[128 * kt:128 * (kt + 1), :])
                    wob = loadp.tile([128, D], BF, tag="ld")
                    eng = nc.scalar if kt % 2 == 0 else nc.vector
                    if eng is nc.scalar:
                        eng.copy(wob[:], wof[:])
                    else:
                        eng.tensor_copy(wob[:], wof[:])
                    nc.sync.dma_start(wred_in[128 * kt:128 * (kt + 1), :], wob[:])
                if sim_cc_as_dma:
                    nc.sync.dma_start(wred_out[:], wred_in[:])
                else:
                    nc.gpsimd.collective_compute(
                        "AllReduce", ALU.add, replica_groups=AR_G,
                        ins=[wred_in.ap().opt()], outs=[wred_out.ap().opt()])
                for hh in range(2):
                    nc.sync.dma_start(
                        weff_bf[:, :, 512 * hh:512 * (hh + 1)],
                        wred_out.ap().rearrange("(kt p) d -> p kt d", p=128)
                        [:, :, 512 * hh:512 * (hh + 1)])

                bias_f = loadp.tile([1, D], F32, tag="ld")
                nc.sync.dma_start(bias_f[:1, :], bias_d[:1, :])
                nc.scalar.copy(bias_bf[:1, :], bias_f[:1, :])



                # ---------- phase 3: load gathered q into SBUF ----------
                # k axis is RANK-MAJOR: rank r's 512 tokens (strips r, 7-r in
                # its local order) occupy k block [512r, 512(r+1)).  The host
                # builds amask in the same permuted k order.
                for r in range(4):
                    nc.sync.dma_start(
                        qT_all[:, :, 512 * r:512 * (r + 1)],
                        qtg_out[r * D * TOK:(r + 1) * D * TOK]
                        .rearrange("(m p t) -> p m t", p=128, m=8))
                # derive q-natural (k-tile major) locally from gathered q^T:
                # one whole-row DMA transpose per d-chunk
                for dch in range(8):
                    nc.scalar.dma_start_transpose(
                        qn_all[:, :, 128 * dch:128 * (dch + 1)],
                        qT_all[:, dch, :])

                # ---------- phase 4+5: attention, out-proj, LN ----------
                # Emission is software-pipelined so each engine's in-order
                # stream never makes PE wait on a later q-tile's softmax:
                #   PE:  sc0 sc1 sc2 PV(s0) op(s0) sc3 PV(s1) op(s1)
                #   DVE: masks0/1, norm0/1, masks2, hT-adds(s0), LN(s0), ...
                E_tiles = {}
                ET_tiles = {}
                for si in range(2):
                    ET_tiles[si] = etp.tile([128, KT, STR], BF, tag="ET",
                                            name=f"ET{si}_{rep}")
                hT_tiles = {}

                def emit_scores(qt):
                    E = ep.tile([128, S], BF, tag="E", name=f"E{qt}_{rep}")
                    E_tiles[qt] = E
                    mk = mkp.tile([128, S], BF, tag="mk", name=f"mk{qt}_{rep}")
                    nc.sync.dma_start(mk[:], amask_d[qt, :, :])
                    for n in range(4):
                        sc = psp.tile([128, 512], F32, tag="ps",
                                      name=f"sc{qt}_{n}_{rep}")
                        for kd in range(8):
                            nc.tensor.matmul(
                                sc[:], qT_my[:, kd, 128 * qt:128 * (qt + 1)],
                                qT_all[:, kd, 512 * n:512 * (n + 1)],
                                start=(kd == 0), stop=(kd == 7))
                        nc.vector.tensor_add(sc[:], sc[:],
                                             mk[:, 512 * n:512 * (n + 1)])
                        nc.scalar.activation(
                            E[:, 512 * n:512 * (n + 1)], sc[:], AF.Exp,
                            bias=0.0, scale=1.0 / 32.0,
                            accum_out=acc[:, 4 * qt + n:4 * qt + n + 1])

                def emit_norm(qt):
                    E = E_tiles[qt]
                    nc.vector.reduce_sum(rinv[:, qt:qt + 1],
                                         acc[:, 4 * qt:4 * qt + 4], axis=AX.X)
                    nc.vector.reciprocal(rinv[:, qt:qt + 1], rinv[:, qt:qt + 1])
                    nc.vector.tensor_scalar_mul(E[:], E[:], rinv[:, qt:qt + 1])
                    if debug:
                        nc.sync.dma_start(dbg["dbg_E"][qt], E[:])
                    si, tl = divmod(qt, 2)
                    nc.scalar.dma_start_transpose(
                        ET_tiles[si][:, :, 128 * tl:128 * (tl + 1)], E[:, :])

                def emit_pv(si):
                    ET = ET_tiles[si]
                    hT = htp.tile([128, 8, STR], BF, tag="hT",
                                  name=f"hT{si}_{rep}")
                    hT_tiles[si] = hT
                    for m in range(8):
                        pv = pvp.tile([128, STR], F32, tag="pv",
                                      name=f"pv{si}_{m}_{rep}")
                        for kt in range(KT):
                            nc.tensor.matmul(
                                pv[:], qn_all[:, kt, 128 * m:128 * (m + 1)],
                                ET[:, kt, :], start=(kt == 0),
                                stop=(kt == KT - 1))
                        nc.vector.tensor_add(
                            hT[:, m, :], pv[:],
                            qT_my[:, m, STR * si:STR * (si + 1)])
                        if debug:
                            nc.sync.dma_start(dbg["dbg_hT"][si, :, m, :],
                                              hT[:, m, :])

                def emit_outproj(si):
                    hT = hT_tiles[si]
                    for tl2 in range(2):
                        qt2 = 2 * si + tl2
                        hps = []
                        for hh in range(2):
                            hp = psp.tile([128, 512], F32, tag="ps",
                                          name=f"hp{qt2}_{hh}_{rep}")
                            for kd in range(8):
                                nc.tensor.matmul(
                                    hp[:], hT[:, kd, 128 * tl2:128 * (tl2 + 1)],
                                    weff_bf[:, kd, 512 * hh:512 * (hh + 1)],
                                    start=(kd == 0), stop=False)
                            nc.tensor.matmul(
                                hp[:], ones1[:1, :],
                                bias_bf[:1, 512 * hh:512 * (hh + 1)],
                                start=False, stop=True)
                            hps.append(hp)
                        # LN: mean/var from sums + sums of squares
                        c0 = 4 * qt2
                        for hh, hp in enumerate(hps):
                            nc.vector.reduce_sum(st[:, c0 + hh:c0 + hh + 1],
                                                 hp[:], axis=AX.X)
                            sqs = sqp.tile([128, 512], F32, tag="sq",
                                           name=f"sq{qt2}_{hh}_{rep}")
                            nc.scalar.activation(
                                sqs[:], hp[:], AF.Square,
                                accum_out=st[:, c0 + 2 + hh:c0 + 3 + hh])
                        mean = smp.tile([128, 4], F32, tag=f"mean{qt2}",
                                        name=f"mean{qt2}_{rep}")
                        nc.vector.tensor_scalar(
                            mean[:, 0:1], st[:, c0:c0 + 1],
                            st[:, c0 + 1:c0 + 2], 1.0 / D,
                            op0=ALU.add, op1=ALU.mult)
                        nc.vector.tensor_scalar(
                            mean[:, 1:2], st[:, c0 + 2:c0 + 3],
                            st[:, c0 + 3:c0 + 4], 1.0 / D,
                            op0=ALU.add, op1=ALU.mult)
                        nc.vector.tensor_tensor(
                            mean[:, 2:3], mean[:, 0:1], mean[:, 0:1],
                            op=ALU.mult)
                        nc.vector.tensor_tensor(
                            mean[:, 2:3], mean[:, 1:2], mean[:, 2:3],
                            op=ALU.subtract)
                        nc.scalar.activation(mean[:, 2:3], mean[:, 2:3],
                                             AF.Sqrt, bias=eps_ap[:, 0:1])
                        nc.vector.reciprocal(mean[:, 2:3], mean[:, 2:3])
                        nc.vector.tensor_scalar(
                            mean[:, 3:4], mean[:, 0:1], mean[:, 2:3], -1.0,
                            op0=ALU.mult, op1=ALU.mult)
                        for hh, hp in enumerate(hps):
                            nc.vector.tensor_scalar(
                                hn_sb[:, qt2, 512 * hh:512 * (hh + 1)], hp[:],
                                mean[:, 2:3], mean[:, 3:4],
                                op0=ALU.mult, op1=ALU.add)

                emit_scores(0)
                emit_scores(1)
                emit_norm(0)
                emit_norm(1)
                emit_scores(2)
                emit_pv(0)
                emit_norm(2)
                emit_scores(3)
                emit_norm(3)
                emit_outproj(0)
                emit_pv(1)
                emit_outproj(1)

                if debug:
                    nc.sync.dma_start(dbg["dbg_qT_my"][:], qT_my[:])
                    nc.sync.dma_start(dbg["dbg_qT_all"][:], qT_all[:])
                    nc.sync.dma_start(dbg["dbg_qn_all"][:], qn_all[:])
                    nc.sync.dma_start(dbg["dbg_weff"][:], weff_bf[:])
                    nc.sync.dma_start(dbg["dbg_hn"][:], hn_sb[:])

                # ---------- phase 6: FFN (token-halves pipelined vs LN) ----------
                hnT = pp.tile([128, 8, TOK], BF, tag="hnT", name=f"hnT{rep}")
                for tt in range(4):
                    nc.scalar.dma_start_transpose(
                        hnT[:, :, 128 * tt:128 * (tt + 1)], hn_sb[:, tt, :])
                for th in range(2):              # token half = strip
                    for m in range(8):
                        f1 = psp.tile([128, STR], F32, tag="ps",
                                      name=f"f1_{rep}_{th}_{m}")
                        for kd in range(8):
                            nc.tensor.matmul(
                                f1[:], wi_bf[:, kd, 128 * m:128 * (m + 1)],
                                hnT[:, kd, STR * th:STR * (th + 1)],
                                start=(kd == 0), stop=(kd == 7))
                        nc.scalar.activation(saT[:, m, STR * th:STR * (th + 1)],
                                             f1[:], AF.Silu)
                    for tt in (2 * th, 2 * th + 1):
                        for hh in range(2):
                            f2 = psp.tile([128, 512], F32, tag="ps",
                                          name=f"f2_{rep}_{tt}_{hh}")
                            for kd in range(8):
                                nc.tensor.matmul(
                                    f2[:], saT[:, kd, 128 * tt:128 * (tt + 1)],
                                    wi_bf[:, kd, 512 * hh:512 * (hh + 1)],
                                    start=(kd == 0), stop=(kd == 7))
                            ob = outp.tile([128, 512], F32, tag="outb",
                                           name=f"ob_{rep}_{tt}_{hh}")
                            nc.scalar.copy(ob[:], f2[:])
                            nc.sync.dma_start(
                                out_d[128 * tt:128 * (tt + 1),
                                      512 * hh:512 * (hh + 1)], ob[:])

    nc.compile()
    return nc


def _get_nc(debug=False, reps=1, sim_cc_as_dma=False):
    key = ("nc", debug, reps, sim_cc_as_dma)
    if key not in _CACHE:
        _CACHE[key] = _build(debug, reps, sim_cc_as_dma)
    return _CACHE[key]


def make_in_maps(x, mask, wi, out_kernel, out_bias):
    """Host-side sharding: build the 8 per-core input dicts."""
    import ml_dtypes

    x = np.ascontiguousarray(x, dtype=np.float32)
    wi = np.ascontiguousarray(wi, dtype=np.float32)
    out_kernel = np.ascontiguousarray(out_kernel, dtype=np.float32)
    bias = np.ascontiguousarray(out_bias, dtype=np.float32).reshape(1, D)
    mask = np.asarray(mask).astype(bool)

    # additive pre-scale mask (0 keep / -1e6 drop), bf16.
    # k columns are permuted to the kernel's rank-major token order:
    # rank r's block = [strip r | strip 7-r].
    perm = np.concatenate([np.r_[STR * s:STR * (s + 1)]
                           for r in range(4) for s in _strips(r)])
    amask_full = np.where(mask, np.float32(0.0), np.float32(MASK_NEG)) \
        .astype(ml_dtypes.bfloat16)[:, perm]

    in_maps = []
    for c in range(NCORES):
        b, j = divmod(c, 4)
        s_a, s_b = _strips(j)
        rows = np.r_[STR * s_a:STR * (s_a + 1), STR * s_b:STR * (s_b + 1)]
        xt = np.ascontiguousarray(x[b, rows, :].T)          # [D, TOK]
        amask = np.ascontiguousarray(
            amask_full[rows, :].reshape(4, 128, S))
        wo = np.ascontiguousarray(out_kernel[D * c:D * (c + 1), :])
        in_maps.append({
            "xt": xt, "wi": wi, "wo": wo, "bias": bias, "amask": amask,
        })
    return in_maps


def assemble_output(results):
    out = np.empty((B, S, D), dtype=np.float32)
    for c in range(NCORES):
        b, j = divmod(c, 4)
        s_a, s_b = _strips(j)
        res = results[c]["out"]
        out[b, STR * s_a:STR * (s_a + 1), :] = res[0:STR, :]
        out[b, STR * s_b:STR * (s_b + 1), :] = res[STR:TOK, :]
    return out


def kernel(x, mask, wi, out_kernel, out_bias, n_heads):
    from concourse.bass_utils import run_bass_kernel_spmd

    assert int(np.asarray(n_heads)) == H
    nc = _get_nc()
    in_maps = make_in_maps(x, mask, wi, out_kernel, out_bias)
    res = run_bass_kernel_spmd(nc, in_maps, core_ids=list(range(NCORES)))
    return assemble_output(res.results)


if __name__ == "__main__":
    # quick self-check against the reference if available
    sys.path.insert(0, "/root/problem")
    import reference

    inputs = {k: np.asarray(v) for k, v in reference.setup_inputs().items()}
    exp = np.asarray(reference.reference(**reference.setup_inputs()))
    act = kernel(**inputs)
    err = np.linalg.norm(act - exp) / np.linalg.norm(exp)
    print("Relative error:", err)



# revision 2
# speedup vs baseline: 357.1922x; 357.1922x over previous
"""Trainium2 (Bass/Tile) 8-core kernel for the dense transformer block.

Math (mirrors the reference):
    q      = x @ wi                       # all heads share wi -> q == k == v
    P      = softmax(mask(q q^T / 32))
    head   = q + P @ q
    h      = head @ W_eff + bias          # cat of identical heads @ out_kernel
                                          # == head @ (sum of the 8 blocks);
                                          # W_eff summed on host.
    hn     = layernorm(h)                 # E[x^2]-E[x]^2 variance, eps=1e-5
    out    = silu(hn @ wi) @ wi

Sharding / dataflow (8 cores, one SPMD NEFF):
  Cores c = 4b + r: batch b, rank r in its 4-core batch group.  The 2048
  tokens of a batch are split into 16 granules of 128 rows; rank r owns
  granules {r, r+4, r+8, r+12} (q slots i=0..3, ascending height).  This
  interleaving makes the causal-skip program UNIFORM across cores: q slot i
  only ever attends to k granules of height < 4(i+1), so score/PV matmuls
  use a fixed narrowing column range [128*(p%4), 512) per k-position p,
  identical on every core.  The diagonal + over-computed blocks are fixed
  by a per-core additive mask input (16 tiles of [128k,128q]).

  Attention runs fully transposed: scores S^T[k,q] accumulate per k-chunk,
  exp -> E^T tiles directly (no transposes), PV -> head^T, out-proj ->
  h^T, LN via ones-matmul partition reductions, FFN -> out^T written
  [D, TOK] and transposed on host.  The only device transposes are the 16
  q^T chunk derivations (xbar DMA transpose, 64 tiles each).

  One AllGather (bf16 q natural layout) per 4-core group, split in two
  0.5 MB halves so the first half's scores overlap the second's gather.
  W_eff = sum of out_kernel blocks is computed on the host (0.01% of
  FLOPs) and shipped bf16; x/wi are shipped bf16 (they feed bf16 matmuls).
"""

import sys

for _p in ("/opt/trn_rl_repo",):
    if _p not in sys.path:
        sys.path.insert(0, _p)

import numpy as np

B, S, D, H = 2, 2048, 1024, 8
NCORES = 8
TOK = 512            # q rows per core
NP = 16              # k positions (128-row granules) per batch
EPS = 1e-5
MASK_NEG = -1.0e6    # pre-scale additive mask value (exp(-1e6/32) == 0)

# position p = 4r + i (rank-major gather order) <-> granule height h = r + 4i
POS_H = [(p % 4) * 4 + p // 4 for p in range(NP)]       # height at position p
# emission order: by gather slot (the AllGather is split per token slot),
# with p=0 first (its sums/PV matmul must cover the full 512 cols: start=True)
P_ORDER = [p for p in range(NP) if p % 4 == 0] + \
          [p for p in range(NP) if p % 4 == 1] + \
          [p for p in range(NP) if p % 4 == 2] + \
          [p for p in range(NP) if p % 4 == 3]

_CACHE = {}


def _width(p):
    """Score/PV column range for k-position p: [128*(p%4), 512)."""
    lo = 128 * (p % 4)
    return lo, 512 - lo


def _build(debug=False, reps=1, sim_cc_as_dma=False):
    import concourse.bacc as bacc
    import concourse.mybir as mybir
    import concourse.tile as tile
    from concourse.replica_groups import maybe_share_collective_output_space

    dt = mybir.dt
    BF, F32 = dt.bfloat16, dt.float32
    AF = mybir.ActivationFunctionType
    ALU = mybir.AluOpType

    nc = bacc.Bacc("TRN2", target_bir_lowering=False, debug=False,
                   num_devices=NCORES)

    # ---------------- I/O (per-core shapes) ----------------
    xt_d = nc.dram_tensor("xt", [D, TOK], BF, kind="ExternalInput")
    wi_d = nc.dram_tensor("wi", [D, D], BF, kind="ExternalInput")
    weff_d = nc.dram_tensor("weff", [D, D], BF, kind="ExternalInput")
    bias_d = nc.dram_tensor("bias", [1, D], BF, kind="ExternalInput")
    wsum_d = nc.dram_tensor("wsum", [1, D], BF, kind="ExternalInput")
    ident_d = nc.dram_tensor("ident", [128, 128], BF, kind="ExternalInput")
    amask_d = nc.dram_tensor("amask", [NP, 128, 128], BF, kind="ExternalInput")
    out_d = nc.dram_tensor("out", [D, TOK], F32, kind="ExternalOutput")
    if debug:
        dbg = {
            "dbg_qn_my": nc.dram_tensor("dbg_qn_my", [128, 4, D], BF, kind="ExternalOutput"),
            "dbg_qT_my": nc.dram_tensor("dbg_qT_my", [128, 8, TOK], BF, kind="ExternalOutput"),
            "dbg_qn_all": nc.dram_tensor("dbg_qn_all", [128, NP, D], BF, kind="ExternalOutput"),
            "dbg_ET": nc.dram_tensor("dbg_ET", [128, NP, TOK], BF, kind="ExternalOutput"),
            "dbg_rinv": nc.dram_tensor("dbg_rinv", [1, TOK], F32, kind="ExternalOutput"),
            "dbg_hT": nc.dram_tensor("dbg_hT", [128, 8, TOK], BF, kind="ExternalOutput"),
            "dbg_hT2": nc.dram_tensor("dbg_hT2", [128, 8, TOK], BF, kind="ExternalOutput"),
        }

    # ---------------- collective buffers -------------------
    AG_G = [[0, 1, 2, 3], [4, 5, 6, 7]]
    QTR = 128 * D                           # elements per rank per AG slot
    qg_in = [nc.dram_tensor(f"qg{h}_in", [QTR], BF) for h in range(4)]
    qg_out = [nc.dram_tensor(
        f"qg{h}_out", [4 * QTR], BF,
        addr_space=maybe_share_collective_output_space("AllGather", AG_G))
        for h in range(4)]

    with tile.TileContext(nc) as tc:
        with (
            tc.tile_pool(name="persist", bufs=1) as pp,
            tc.tile_pool(name="ps", bufs=3, space="PSUM") as psp,
            tc.tile_pool(name="acc", bufs=1, space="PSUM") as accp,
            tc.tile_pool(name="bc", bufs=1, space="PSUM") as bcp,
            tc.tile_pool(name="qTps", bufs=3, space="PSUM") as qtpsp,
            tc.tile_pool(name="qTc", bufs=4) as qtcp,
            tc.tile_pool(name="sq", bufs=2) as sqp,
            tc.tile_pool(name="outb", bufs=3) as outp,
            tc.tile_pool(name="small", bufs=1) as smp,
        ):
            # persistent SBUF tensors
            wi_bf = pp.tile([128, 8, D], BF, tag="wi_bf")
            weff_bf = pp.tile([128, 8, D], BF, tag="weff_bf")
            xt_sb = pp.tile([128, 8, TOK], BF, tag="xt_sb")
            amask_sb = pp.tile([128, NP, 128], BF, tag="amask_sb")
            bias_sb = pp.tile([1, D], BF, tag="bias_sb")

            ones_k = smp.tile([128, 1], BF, tag="ones_k")
            ones_q = smp.tile([1, 128], F32, tag="ones_q")
            ones_row = smp.tile([1, 512], BF, tag="ones_row")
            eps_ap = smp.tile([1, 1], F32, tag="eps_ap")
            nc.vector.memset(ones_k[:], 1.0)
            nc.vector.memset(ones_q[:], 1.0)
            nc.vector.memset(ones_row[:], 1.0)
            nc.vector.memset(eps_ap[:], EPS)

            wsum_sb = smp.tile([1, D], BF, tag="wsum_sb")
            ident_sb = smp.tile([128, 128], BF, tag="ident_sb")

            # ---------------- loads (x, wi only: they gate q; the rest is
            # emitted on the ACT queue mid-q so its DMA slots in later).
            # Token-half A of x first, then wi chunks: the first q groups
            # (token slots 0,1 -> AG half 0) become runnable earliest.
            nc.sync.dma_start(ident_sb[:], ident_d[:, :])
            nc.sync.dma_start(
                xt_sb[:, :, 0:256],
                xt_d[:, 0:256].rearrange("(k p) t -> p k t", p=128))
            for c4 in range(4):
                nc.sync.dma_start(
                    wi_bf[:, 2 * c4:2 * (c4 + 1), :],
                    wi_d[256 * c4:256 * (c4 + 1), :]
                    .rearrange("(k p) d -> p k d", p=128))
            nc.sync.dma_start(
                xt_sb[:, :, 256:512],
                xt_d[:, 256:512].rearrange("(k p) t -> p k t", p=128))

            def emit_late_loads(stage):
                if stage == 0:
                    nc.scalar.dma_start(
                        amask_sb[:], amask_d.ap().rearrange("t p q -> p t q"))
                elif stage == 1:
                    nc.scalar.dma_start(bias_sb[:], bias_d[:, :])
                    nc.scalar.dma_start(wsum_sb[:], wsum_d[:, :])
                else:
                    for c2 in range(2):
                        nc.scalar.dma_start(
                            weff_bf[:, 4 * c2:4 * (c2 + 1), :],
                            weff_d[512 * c2:512 * (c2 + 1), :]
                            .rearrange("(k p) d -> p k d", p=128))

            for rep in range(reps):
                sfx = f"_{rep}"
                # ---------- phase 1: q = x @ wi (natural layout) ----------
                # AG half g launches as soon as its two q slots are done, so
                # the first gather overlaps the rest of the q matmuls.
                qn_my = pp.tile([128, 4, D], BF, tag="qn_my", name="qn_my" + sfx)

                def emit_ag(g):
                    nc.sync.dma_start(
                        qg_in[g].ap().rearrange("(p d) -> p d", p=128),
                        qn_my[:, g, :])
                    if sim_cc_as_dma:
                        for r in range(4):
                            nc.sync.dma_start(
                                qg_out[g][r * QTR:(r + 1) * QTR],
                                qg_in[g][:])
                    else:
                        nc.gpsimd.collective_compute(
                            "AllGather", ALU.bypass, replica_groups=AG_G,
                            ins=[qg_in[g].ap().opt()],
                            outs=[qg_out[g].ap().opt()])

                qn_all = pp.tile([128, NP, D], BF, tag="qn_all",
                                 name="qn_all" + sfx)

                def emit_unpack(g):
                    # gathered q natural -> qn_all; position p = 4r+i.
                    # SWDGE (gpsimd) queue: sits right behind its AllGather,
                    # off the SP/ACT trigger paths.
                    for r in range(4):
                        nc.sync.dma_start(
                            qn_all[:, 4 * r + g:4 * r + g + 1, :],
                            qg_out[g][r * QTR:(r + 1) * QTR]
                            .rearrange("(p d) -> p d", p=128))


                qT_my = pp.tile([128, 8, TOK], BF, tag="qT_my",
                                name="qT_my" + sfx)

                def emit_tmy(tt):
                    # own q^T via PE transposes (keeps PE warm pre-gather)
                    tps = qtpsp.tile([128, 8, 128], BF, tag="qTps",
                                     name=f"tmy{tt}{sfx}")
                    for i8 in range(8):
                        nc.tensor.transpose(
                            tps[:, i8, :],
                            qn_my[:, tt, 128 * i8:128 * (i8 + 1)],
                            ident_sb[:])
                    eng = nc.scalar if tt % 2 == 0 else nc.vector
                    if eng is nc.scalar:
                        eng.copy(qT_my[:, :, 128 * tt:128 * (tt + 1)], tps[:])
                    else:
                        eng.tensor_copy(qT_my[:, :, 128 * tt:128 * (tt + 1)],
                                        tps[:])

                for tt in range(4):
                    for hh in range(2):
                        qn_ps = psp.tile([128, 512], F32, tag="ps",
                                         name=f"qn{tt}_{hh}{sfx}")
                        for kd in range(8):
                            nc.tensor.matmul(
                                qn_ps[:],
                                xt_sb[:, kd, 128 * tt:128 * (tt + 1)],
                                wi_bf[:, kd, 512 * hh:512 * (hh + 1)],
                                start=(kd == 0), stop=(kd == 7))
                        nc.scalar.copy(
                            qn_my[:, tt, 512 * hh:512 * (hh + 1)], qn_ps[:])
                    emit_tmy(tt)
                    emit_ag(tt)
                    emit_unpack(tt)
                    if rep == 0 and tt in (0, 2, 3):
                        emit_late_loads({0: 0, 2: 1, 3: 2}[tt])


                # ---------- phase 3: scores^T + exp + softmax sums ----------
                # one PSUM bank holds all three [1,512] accumulators at
                # partition offsets 0/32/64 (softmax sums, LN sum, LN sumsq)
                ET = pp.tile([128, NP, TOK], BF, tag="ET", name="ET" + sfx)
                acc_ps = accp.tile([96, 512], F32, tag="acc",
                                   name="acc_ps" + sfx)
                sc_tiles = {}

                def emit_qtc(p):
                    # q^T chunk via 8 PE tile-transposes + one wide copy
                    tps = qtpsp.tile([128, 8, 128], BF, tag="qTps",
                                     name=f"tqc{p}{sfx}")
                    for i8 in range(8):
                        nc.tensor.transpose(
                            tps[:, i8, :],
                            qn_all[:, p, 128 * i8:128 * (i8 + 1)],
                            ident_sb[:])
                    qTc = qtcp.tile([128, 8, 128], BF, tag="qTc",
                                    name=f"qTc{p}{sfx}")
                    nc.vector.tensor_copy(qTc[:], tps[:])
                    return qTc

                def emit_score(p, qTc):
                    lo, w = _width(p)
                    sc = psp.tile([128, 512], F32, tag="ps",
                                  name=f"sc{p}{sfx}")
                    sc_tiles[p] = sc
                    for kd in range(8):
                        nc.tensor.matmul(
                            sc[:, :w], qTc[:, kd, :],
                            qT_my[:, kd, lo:512],
                            start=(kd == 0), stop=False)
                    # mask add on PE: sc[:, :128] += I^T @ amask (53 ns)
                    nc.tensor.matmul(
                        sc[:, 0:128], ident_sb[:], amask_sb[:, p, :],
                        start=False, stop=True)
                    nc.scalar.activation(
                        ET[:, p, lo:512], sc[:, :w], AF.Exp,
                        bias=0.0, scale=1.0 / 32.0)

                def emit_sum(p, is_first, is_last):
                    lo, w = _width(p)
                    nc.tensor.matmul(
                        acc_ps[0:1, lo:512], ones_k[:, :], ET[:, p, lo:512],
                        start=is_first, stop=is_last)

                # transposes run two chunks ahead of their scores (they are PE
                # work and fill the stream while the chunk's copy completes);
                # the sums-MM for chunk n lands after the scores of chunk n+1.
                order = P_ORDER
                qtcs = {p: emit_qtc(p) for p in order[:3]}
                for n, p in enumerate(order):
                    if n + 3 < len(order):
                        qtcs[order[n + 3]] = emit_qtc(order[n + 3])
                    emit_score(p, qtcs.pop(p))
                    if n > 0:
                        emit_sum(order[n - 1], order[n - 1] == order[0], False)
                emit_sum(order[-1], False, True)

                # 1/sums -> broadcast across partitions
                rinv = smp.tile([1, 512], F32, tag="rinv", name="rinv" + sfx)
                nc.vector.reciprocal(rinv[:], acc_ps[0:1, :])
                if debug:
                    nc.sync.dma_start(dbg["dbg_rinv"][:], rinv[:])
                rb_ps = bcp.tile([128, 512], F32, tag="bc", name="rb_ps" + sfx)
                nc.tensor.matmul(rb_ps[:], ones_q[:, :], rinv[:],
                                 start=True, stop=True)
                rb_sb = smp.tile([128, 512], F32, tag="rb_sb",
                                 name="rb_sb" + sfx)
                nc.scalar.copy(rb_sb[:], rb_ps[:])

                # ---------- phase 4: PV -> head^T ----------
                hT = pp.tile([128, 8, TOK], BF, tag="hT", name="hT" + sfx)
                for m in range(8):
                    pv = psp.tile([128, 512], F32, tag="ps",
                                  name=f"pv{m}{sfx}")
                    for p in range(NP):
                        lo, w = _width(p)
                        nc.tensor.matmul(
                            pv[:, lo:512], qn_all[:, p, 128 * m:128 * (m + 1)],
                            ET[:, p, lo:512],
                            start=(p == 0), stop=(p == NP - 1))
                    nc.vector.tensor_tensor(hT[:, m, :], pv[:], rb_sb[:],
                                            op=ALU.mult)
                    nc.vector.tensor_tensor(hT[:, m, :], hT[:, m, :],
                                            qT_my[:, m, :], op=ALU.add)

                # ---------- phase 5: out-proj^T + LN stats ----------
                hT2 = pp.tile([128, 8, TOK], BF, tag="hT2", name="hT2" + sfx)
                ln_sum = acc_ps[32:33, :]
                ln_sq = acc_ps[64:65, :]
                sq_tiles = {}

                def emit_oproj(mo):
                    hp = psp.tile([128, 512], F32, tag="ps",
                                  name=f"hp{mo}{sfx}")
                    for kd in range(8):
                        nc.tensor.matmul(
                            hp[:], weff_bf[:, kd, 128 * mo:128 * (mo + 1)],
                            hT[:, kd, :], start=(kd == 0), stop=False)
                    # bias add on PE: hp += bias_chunk (x) ones_row
                    nc.tensor.matmul(
                        hp[:], bias_sb[:, 128 * mo:128 * (mo + 1)],
                        ones_row[:], start=False, stop=True)
                    nc.scalar.copy(hT2[:, mo, :], hp[:])
                    sq = sqp.tile([128, 512], BF, tag="sq", name=f"sq{mo}{sfx}")
                    sq_tiles[mo] = sq
                    nc.scalar.activation(sq[:], hp[:], AF.Square)

                def emit_lnsum(mo):
                    nc.tensor.matmul(ln_sum, ones_k[:, :], hT2[:, mo, :],
                                     start=(mo == 0), stop=(mo == 7))
                    nc.tensor.matmul(ln_sq, ones_k[:, :], sq_tiles[mo][:],
                                     start=(mo == 0), stop=(mo == 7))

                for mo in range(8):
                    emit_oproj(mo)
                    if mo > 0:
                        emit_lnsum(mo - 1)
                emit_lnsum(7)

                # LN scalars on [1,512].  LayerNorm is folded into FFN1:
                #   f1 = rstd * (wi^T @ hT2 - wsum (x) mean),  wsum = wi.sum(0)
                # so FFN1's matmuls start straight from hT2 (no barrier).
                stat = smp.tile([1, 512 * 4], F32, tag="stat",
                                name="stat" + sfx)
                mean, msq, var, rstd = (stat[:, 512 * k:512 * (k + 1)]
                                        for k in range(4))
                nc.vector.tensor_scalar(mean, ln_sum, 1.0 / D, None,
                                        op0=ALU.mult)
                nc.vector.tensor_scalar(msq, ln_sq, 1.0 / D, None,
                                        op0=ALU.mult)
                nc.vector.tensor_tensor(var, mean, mean, op=ALU.mult)
                nc.vector.tensor_tensor(var, msq, var, op=ALU.subtract)
                nc.scalar.activation(var, var, AF.Sqrt, bias=eps_ap[:, 0:1])
                nc.vector.reciprocal(rstd, var)
                nmean = smp.tile([1, 512], BF, tag="nmean", name="nmean" + sfx)
                nc.vector.tensor_scalar(nmean[:], mean, -1.0, None,
                                        op0=ALU.mult)
                rstd_ps = bcp.tile([128, 512], F32, tag="bc",
                                   name="rstd_ps" + sfx)
                nc.tensor.matmul(rstd_ps[:], ones_q[:, :], rstd,
                                 start=True, stop=True)
                rstd_b = smp.tile([128, 512], F32, tag="rstd_b",
                                  name="rstd_b" + sfx)
                nc.scalar.copy(rstd_b[:], rstd_ps[:])

                # ---------- phase 6: FFN (transposed, LN fused into FFN1) ----
                saT = pp.tile([128, 8, TOK], BF, tag="saT", name="saT" + sfx)
                for m in range(8):
                    f1 = psp.tile([128, 512], F32, tag="ps",
                                  name=f"f1_{m}{sfx}")
                    for mo in range(8):
                        nc.tensor.matmul(
                            f1[:], wi_bf[:, mo, 128 * m:128 * (m + 1)],
                            hT2[:, mo, :], start=(mo == 0), stop=False)
                    nc.tensor.matmul(
                        f1[:], wsum_sb[:, 128 * m:128 * (m + 1)], nmean[:],
                        start=False, stop=True)
                    f1s = sqp.tile([128, 512], F32, tag="f1s",
                                   name=f"f1s{m}{sfx}")
                    nc.vector.tensor_tensor(f1s[:], f1[:], rstd_b[:],
                                            op=ALU.mult)
                    nc.scalar.activation(saT[:, m, :], f1s[:], AF.Silu)
                for mo in range(8):
                    f2 = psp.tile([128, 512], F32, tag="ps",
                                  name=f"f2_{mo}{sfx}")
                    for m in range(8):
                        nc.tensor.matmul(
                            f2[:], wi_bf[:, m, 128 * mo:128 * (mo + 1)],
                            saT[:, m, :], start=(m == 0), stop=(m == 7))
                    ob = outp.tile([128, 512], F32, tag="outb",
                                   name=f"ob{mo}{sfx}")
                    nc.scalar.copy(ob[:], f2[:])
                    nc.sync.dma_start(
                        out_d[128 * mo:128 * (mo + 1), :], ob[:])

                if debug:
                    nc.sync.dma_start(dbg["dbg_qn_my"][:], qn_my[:])
                    nc.sync.dma_start(dbg["dbg_qT_my"][:], qT_my[:])
                    nc.sync.dma_start(dbg["dbg_qn_all"][:], qn_all[:])
                    nc.sync.dma_start(dbg["dbg_ET"][:], ET[:])
                    nc.sync.dma_start(dbg["dbg_hT"][:], hT[:])
                    nc.sync.dma_start(dbg["dbg_hT2"][:], hT2[:])

    nc.compile()
    return nc


def _get_nc(debug=False, reps=1, sim_cc_as_dma=False):
    key = ("nc", debug, reps, sim_cc_as_dma)
    if key not in _CACHE:
        _CACHE[key] = _build(debug, reps, sim_cc_as_dma)
    return _CACHE[key]


def _granules(r):
    return [r + 4 * i for i in range(4)]


def make_in_maps(x, mask, wi, out_kernel, out_bias):
    """Host-side sharding: build the 8 per-core input dicts."""
    import ml_dtypes

    BFh = ml_dtypes.bfloat16
    x = np.asarray(x, dtype=np.float32)
    wi_bf = np.ascontiguousarray(np.asarray(wi, np.float32).astype(BFh))
    wsum = np.asarray(wi, np.float32).astype(BFh).astype(np.float32) \
        .sum(axis=0).reshape(1, D).astype(BFh)
    weff = np.asarray(out_kernel, np.float32).reshape(H, D, D).sum(axis=0)
    weff_bf = np.ascontiguousarray(weff.astype(BFh))
    bias = np.ascontiguousarray(
        np.asarray(out_bias, np.float32).reshape(1, D).astype(BFh))
    ident = np.eye(128, dtype=BFh)
    mask = np.asarray(mask).astype(bool)
    amadd = np.where(mask, np.float32(0.0), np.float32(MASK_NEG))

    in_maps = []
    for c in range(NCORES):
        b, r = divmod(c, 4)
        gs = _granules(r)
        rows = np.concatenate([np.r_[128 * g:128 * (g + 1)] for g in gs])
        xt = np.ascontiguousarray(x[b, rows, :].T.astype(BFh))  # [D, TOK]
        am = np.empty((NP, 128, 128), dtype=BFh)
        for p in range(NP):
            h = POS_H[p]           # k granule height at position p
            i = p % 4              # masked q slot
            gq = r + 4 * i         # this core's q granule in slot i
            am[p] = amadd[128 * gq:128 * (gq + 1),
                          128 * h:128 * (h + 1)].T.astype(BFh)
        in_maps.append({
            "xt": xt, "wi": wi_bf, "weff": weff_bf, "bias": bias,
            "wsum": wsum, "ident": ident, "amask": am,
        })
    return in_maps


def check_mask_causal(mask):
    """The compiled program skips k granules of height >= 4*(slot+1); that is
    only sound if those blocks are fully masked for every q row that might
    occupy the slot, i.e. for the causal (tril) mask or stricter."""
    mask = np.asarray(mask).astype(bool)
    for i in range(4):
        for r in range(4):
            gq = r + 4 * i
            if mask[128 * gq:128 * (gq + 1), 128 * 4 * (i + 1):].any():
                return False
    return True


def assemble_output(results):
    out = np.empty((B, S, D), dtype=np.float32)
    for c in range(NCORES):
        b, r = divmod(c, 4)
        res = results[c]["out"]                      # [D, TOK]
        for i, g in enumerate(_granules(r)):
            out[b, 128 * g:128 * (g + 1), :] = res[:, 128 * i:128 * (i + 1)].T
    return out


def kernel(x, mask, wi, out_kernel, out_bias, n_heads):
    from concourse.bass_utils import run_bass_kernel_spmd

    assert int(np.asarray(n_heads)) == H
    assert check_mask_causal(mask), "compiled program requires a causal mask"
    nc = _get_nc()
    in_maps = make_in_maps(x, mask, wi, out_kernel, out_bias)
    res = run_bass_kernel_spmd(nc, in_maps, core_ids=list(range(NCORES)))
    return assemble_output(res.results)


if __name__ == "__main__":
    sys.path.insert(0, "/root/problem")
    import reference

    inputs = {k: np.asarray(v) for k, v in reference.setup_inputs().items()}
    exp = np.asarray(reference.reference(**reference.setup_inputs()))
    act = kernel(**inputs)
    err = np.linalg.norm(act - exp) / np.linalg.norm(exp)
    print("Relative error:", err)


# revision 6
# speedup vs baseline: 361.1897x; 1.0112x over previous
"""Trainium2 (Bass/Tile) 8-core kernel for the dense transformer block.

Math (mirrors the reference):
    q      = x @ wi                       # all heads share wi -> q == k == v
    P      = softmax(mask(q q^T / 32))
    head   = q + P @ q
    h      = head @ W_eff + bias          # cat of identical heads @ out_kernel
                                          # == head @ (sum of the 8 blocks);
                                          # W_eff summed on host.
    hn     = layernorm(h)                 # E[x^2]-E[x]^2 variance, eps=1e-5
    out    = silu(hn @ wi) @ wi

Sharding / dataflow (8 cores, one SPMD NEFF):
  Cores c = 4b + r: batch b, rank r in its 4-core batch group.  The 2048
  tokens of a batch are split into 16 granules of 128 rows; rank r owns
  granules {r, r+4, r+8, r+12} (q slots i=0..3, ascending height).  This
  interleaving makes the causal-skip program UNIFORM across cores: q slot i
  only ever attends to k granules of height < 4(i+1), so score/PV matmuls
  use a fixed narrowing column range [128*(p%4), 512) per k-position p,
  identical on every core.  The diagonal + over-computed blocks are fixed
  by a per-core additive mask input (16 tiles of [128k,128q]).

  Attention runs fully transposed: scores S^T[k,q] accumulate per k-chunk
  (additive mask applied as one extra PE matmul I^T @ amask), exp -> E^T
  tiles directly, PV -> head^T, out-proj -> h^T (bias via outer-product
  matmul), LN via ones-matmul partition reductions with LayerNorm folded
  into FFN1 (f1 = rstd*(wi^T @ h^T - wsum (x) mean), wsum = wi.sum(0)
  shipped from host), FFN -> out^T written [D, TOK], transposed on host.
  All q^T derivations (own + gathered chunks) use PE tile-transposes via
  an identity input — no xbar DMA transposes anywhere (they serialize
  against every DMA copy on the xbar-mode transition).

  The AllGather (bf16 q natural layout) per 4-core group is split into
  four 256 KB per-slot gathers so early k-chunks' scores overlap the
  later slots' gathers and the tail of the q projection.  W_eff = sum of
  out_kernel blocks is computed on the host (0.01% of FLOPs) and shipped
  bf16; x/wi are shipped bf16 (they feed bf16 matmuls anyway).  A burst
  of tiny matmuls at kernel start keeps the PE clock-gate warm through
  the input-load window.
"""

import sys

for _p in ("/opt/trn_rl_repo",):
    if _p not in sys.path:
        sys.path.insert(0, _p)

import numpy as np

B, S, D, H = 2, 2048, 1024, 8
NCORES = 8
TOK = 512            # q rows per core
NP = 16              # k positions (128-row granules) per batch
EPS = 1e-5
MASK_NEG = -1.0e6    # pre-scale additive mask value (exp(-1e6/32) == 0)

# position p = 4r + i (rank-major gather order) <-> granule height h = r + 4i
POS_H = [(p % 4) * 4 + p // 4 for p in range(NP)]       # height at position p
# emission order: by gather slot (the AllGather is split per token slot),
# with p=0 first (its sums/PV matmul must cover the full 512 cols: start=True)
P_ORDER = [p for p in range(NP) if p % 4 == 0] + \
          [p for p in range(NP) if p % 4 == 1] + \
          [p for p in range(NP) if p % 4 == 2] + \
          [p for p in range(NP) if p % 4 == 3]

_CACHE = {}


def _width(p):
    """Score/PV column range for k-position p: [128*(p%4), 512)."""
    lo = 128 * (p % 4)
    return lo, 512 - lo


def _build(debug=False, reps=1, sim_cc_as_dma=False):
    import concourse.bacc as bacc
    import concourse.mybir as mybir
    import concourse.tile as tile
    from concourse.replica_groups import maybe_share_collective_output_space

    dt = mybir.dt
    BF, F32 = dt.bfloat16, dt.float32
    AF = mybir.ActivationFunctionType
    ALU = mybir.AluOpType

    nc = bacc.Bacc("TRN2", target_bir_lowering=False, debug=False,
                   num_devices=NCORES)

    # ---------------- I/O (per-core shapes) ----------------
    xt_d = nc.dram_tensor("xt", [D, TOK], BF, kind="ExternalInput")
    wi_d = nc.dram_tensor("wi", [D, D], BF, kind="ExternalInput")
    weff_d = nc.dram_tensor("weff", [D, D], BF, kind="ExternalInput")
    bias_d = nc.dram_tensor("bias", [1, D], BF, kind="ExternalInput")
    wsum_d = nc.dram_tensor("wsum", [1, D], BF, kind="ExternalInput")
    ident_d = nc.dram_tensor("ident", [128, 128], BF, kind="ExternalInput")
    amask_d = nc.dram_tensor("amask", [NP, 128, 128], BF, kind="ExternalInput")
    out_d = nc.dram_tensor("out", [D, TOK], F32, kind="ExternalOutput")
    if debug:
        dbg = {
            "dbg_qn_my": nc.dram_tensor("dbg_qn_my", [128, 4, D], BF, kind="ExternalOutput"),
            "dbg_qT_my": nc.dram_tensor("dbg_qT_my", [128, 8, TOK], BF, kind="ExternalOutput"),
            "dbg_qn_all": nc.dram_tensor("dbg_qn_all", [128, NP, D], BF, kind="ExternalOutput"),
            "dbg_ET": nc.dram_tensor("dbg_ET", [128, NP, TOK], BF, kind="ExternalOutput"),
            "dbg_rinv": nc.dram_tensor("dbg_rinv", [1, TOK], F32, kind="ExternalOutput"),
            "dbg_hT": nc.dram_tensor("dbg_hT", [128, 8, TOK], BF, kind="ExternalOutput"),
            "dbg_hT2": nc.dram_tensor("dbg_hT2", [128, 8, TOK], BF, kind="ExternalOutput"),
        }

    # ---------------- collective buffers -------------------
    AG_G = [[0, 1, 2, 3], [4, 5, 6, 7]]
    QTR = 128 * D                           # elements per rank per AG slot
    qg_in = [nc.dram_tensor(f"qg{h}_in", [QTR], BF) for h in range(4)]
    qg_out = [nc.dram_tensor(
        f"qg{h}_out", [4 * QTR], BF,
        addr_space=maybe_share_collective_output_space("AllGather", AG_G))
        for h in range(4)]

    with tile.TileContext(nc) as tc:
        with (
            tc.tile_pool(name="persist", bufs=1) as pp,
            tc.tile_pool(name="ps", bufs=3, space="PSUM") as psp,
            tc.tile_pool(name="acc", bufs=1, space="PSUM") as accp,
            tc.tile_pool(name="bc", bufs=1, space="PSUM") as bcp,
            tc.tile_pool(name="qTps", bufs=3, space="PSUM") as qtpsp,
            tc.tile_pool(name="qTc", bufs=4) as qtcp,
            tc.tile_pool(name="sq", bufs=2) as sqp,
            tc.tile_pool(name="outb", bufs=3) as outp,
            tc.tile_pool(name="small", bufs=1) as smp,
        ):
            # persistent SBUF tensors.  wi/xt are split into per-DMA tiles:
            # Tile tracks deps per tile, so q matmuls gate on their own
            # chunk's load instead of the whole tensor.
            wi_t = [pp.tile([128, 2, D], BF, tag=f"wi_t{c}",
                            name=f"wi_t{c}") for c in range(4)]
            weff_bf = pp.tile([128, 8, D], BF, tag="weff_bf")
            xt_t = [pp.tile([128, 8, 256], BF, tag=f"xt_t{c}",
                            name=f"xt_t{c}") for c in range(2)]
            amask_sb = pp.tile([128, NP, 128], BF, tag="amask_sb")
            bias_sb = pp.tile([1, D], BF, tag="bias_sb")

            ones_k = smp.tile([128, 1], BF, tag="ones_k")
            ones_q = smp.tile([1, 128], F32, tag="ones_q")
            ones_row = smp.tile([1, 512], BF, tag="ones_row")
            eps_ap = smp.tile([1, 1], F32, tag="eps_ap")
            nc.vector.memset(ones_k[:], 1.0)
            nc.vector.memset(ones_q[:], 1.0)
            nc.vector.memset(ones_row[:], 1.0)
            nc.vector.memset(eps_ap[:], EPS)

            wsum_sb = smp.tile([1, D], BF, tag="wsum_sb")
            ident_sb = smp.tile([128, 128], BF, tag="ident_sb")

            # ---------------- loads (x, wi only: they gate q; the rest is
            # emitted on the ACT queue mid-q so its DMA slots in later).
            # Token-half A of x first, then wi chunks: the first q groups
            # (token slots 0,1 -> AG half 0) become runnable earliest.
            nc.sync.dma_start(ident_sb[:], ident_d[:, :])
            nc.sync.dma_start(
                xt_t[0][:], xt_d[:, 0:256].rearrange("(k p) t -> p k t", p=128))
            for c4 in range(4):
                nc.sync.dma_start(
                    wi_t[c4][:],
                    wi_d[256 * c4:256 * (c4 + 1), :]
                    .rearrange("(k p) d -> p k d", p=128))
            nc.sync.dma_start(
                xt_t[1][:],
                xt_d[:, 256:512].rearrange("(k p) t -> p k t", p=128))

            def wi_ap(kd, c0, c1):
                return wi_t[kd // 2][:, kd % 2, c0:c1]

            def xt_ap(kd, t0, t1):
                return xt_t[t0 // 256][:, kd, t0 % 256:(t1 - 1) % 256 + 1]

            def emit_late_loads(stage):
                if stage == 0:
                    nc.scalar.dma_start(
                        amask_sb[:], amask_d.ap().rearrange("t p q -> p t q"))
                elif stage == 1:
                    nc.scalar.dma_start(bias_sb[:], bias_d[:, :])
                    nc.scalar.dma_start(wsum_sb[:], wsum_d[:, :])
                else:
                    for c2 in range(2):
                        nc.scalar.dma_start(
                            weff_bf[:, 4 * c2:4 * (c2 + 1), :],
                            weff_d[512 * c2:512 * (c2 + 1), :]
                            .rearrange("(k p) d -> p k d", p=128))

            for rep in range(reps):
                sfx = f"_{rep}"
                # ---------- phase 1: q = x @ wi (natural layout) ----------
                # AG half g launches as soon as its two q slots are done, so
                # the first gather overlaps the rest of the q matmuls.
                qn_my = pp.tile([128, 4, D], BF, tag="qn_my", name="qn_my" + sfx)

                def emit_ag(g):
                    nc.sync.dma_start(
                        qg_in[g].ap().rearrange("(p d) -> p d", p=128),
                        qn_my[:, g, :])
                    if sim_cc_as_dma:
                        for r in range(4):
                            nc.sync.dma_start(
                                qg_out[g][r * QTR:(r + 1) * QTR],
                                qg_in[g][:])
                    else:
                        nc.gpsimd.collective_compute(
                            "AllGather", ALU.bypass, replica_groups=AG_G,
                            ins=[qg_in[g].ap().opt()],
                            outs=[qg_out[g].ap().opt()])

                qn_all = pp.tile([128, NP, D], BF, tag="qn_all",
                                 name="qn_all" + sfx)

                def emit_unpack(g):
                    # gathered q natural -> qn_all; position p = 4r+i.
                    # SWDGE (gpsimd) queue: sits right behind its AllGather,
                    # off the SP/ACT trigger paths.
                    for r in range(4):
                        nc.sync.dma_start(
                            qn_all[:, 4 * r + g:4 * r + g + 1, :],
                            qg_out[g][r * QTR:(r + 1) * QTR]
                            .rearrange("(p d) -> p d", p=128))


                qT_my = pp.tile([128, 8, TOK], BF, tag="qT_my",
                                name="qT_my" + sfx)

                def emit_tmy(tt):
                    # own q^T via PE transposes (keeps PE warm pre-gather)
                    tps = qtpsp.tile([128, 8, 128], BF, tag="qTps",
                                     name=f"tmy{tt}{sfx}")
                    for i8 in range(8):
                        nc.tensor.transpose(
                            tps[:, i8, :],
                            qn_my[:, tt, 128 * i8:128 * (i8 + 1)],
                            ident_sb[:])
                    eng = nc.scalar if tt % 2 == 0 else nc.vector
                    if eng is nc.scalar:
                        eng.copy(qT_my[:, :, 128 * tt:128 * (tt + 1)], tps[:])
                    else:
                        eng.tensor_copy(qT_my[:, :, 128 * tt:128 * (tt + 1)],
                                        tps[:])

                acc_ps = accp.tile([96, 512], F32, tag="acc",
                                   name="acc_ps" + sfx)
                if rep == 0:
                    # keep the PE clock warm through the input-load window
                    for wdx in range(100):
                        nc.tensor.matmul(acc_ps[64:65, 0:16],
                                         ones_k[0:1, 0:1],
                                         ones_row[0:1, 0:16],
                                         start=True, stop=True)

                for tt in range(4):
                    for hh in range(2):
                        qn_ps = psp.tile([128, 512], F32, tag="ps",
                                         name=f"qn{tt}_{hh}{sfx}")
                        for kd in range(8):
                            nc.tensor.matmul(
                                qn_ps[:],
                                xt_ap(kd, 128 * tt, 128 * (tt + 1)),
                                wi_ap(kd, 512 * hh, 512 * (hh + 1)),
                                start=(kd == 0), stop=(kd == 7))
                        nc.scalar.copy(
                            qn_my[:, tt, 512 * hh:512 * (hh + 1)], qn_ps[:])
                    emit_tmy(tt)
                    emit_ag(tt)
                    emit_unpack(tt)
                    if rep == 0 and tt in (0, 2, 3):
                        emit_late_loads({0: 0, 2: 1, 3: 2}[tt])


                # ---------- phase 3: scores^T + exp + softmax sums ----------
                # one PSUM bank holds all three [1,512] accumulators at
                # partition offsets 0/32/64 (softmax sums, LN sum, LN sumsq)
                ET = pp.tile([128, NP, TOK], BF, tag="ET", name="ET" + sfx)
                sc_tiles = {}

                def emit_qtc(p):
                    # q^T chunk via 8 PE tile-transposes + one wide copy
                    tps = qtpsp.tile([128, 8, 128], BF, tag="qTps",
                                     name=f"tqc{p}{sfx}")
                    for i8 in range(8):
                        nc.tensor.transpose(
                            tps[:, i8, :],
                            qn_all[:, p, 128 * i8:128 * (i8 + 1)],
                            ident_sb[:])
                    qTc = qtcp.tile([128, 8, 128], BF, tag="qTc",
                                    name=f"qTc{p}{sfx}")
                    nc.vector.tensor_copy(qTc[:], tps[:])
                    return qTc

                def emit_score(p, qTc):
                    lo, w = _width(p)
                    sc = psp.tile([128, 512], F32, tag="ps",
                                  name=f"sc{p}{sfx}")
                    sc_tiles[p] = sc
                    for kd in range(8):
                        nc.tensor.matmul(
                            sc[:, :w], qTc[:, kd, :],
                            qT_my[:, kd, lo:512],
                            start=(kd == 0), stop=False)
                    # mask add on PE: sc[:, :128] += I^T @ amask (53 ns)
                    nc.tensor.matmul(
                        sc[:, 0:128], ident_sb[:], amask_sb[:, p, :],
                        start=False, stop=True)
                    nc.scalar.activation(
                        ET[:, p, lo:512], sc[:, :w], AF.Exp,
                        bias=0.0, scale=1.0 / 32.0)

                def emit_sum(p, is_first, is_last):
                    lo, w = _width(p)
                    nc.tensor.matmul(
                        acc_ps[0:1, lo:512], ones_k[:, :], ET[:, p, lo:512],
                        start=is_first, stop=is_last)

                # transposes run two chunks ahead of their scores (they are PE
                # work and fill the stream while the chunk's copy completes);
                # the sums-MM for chunk n lands after the scores of chunk n+1.
                order = P_ORDER
                qtcs = {p: emit_qtc(p) for p in order[:3]}
                for n, p in enumerate(order):
                    if n + 3 < len(order):
                        qtcs[order[n + 3]] = emit_qtc(order[n + 3])
                    emit_score(p, qtcs.pop(p))
                    if n > 1:
                        emit_sum(order[n - 2], order[n - 2] == order[0], False)
                emit_sum(order[-2], False, False)
                emit_sum(order[-1], False, True)

                # 1/sums -> broadcast across partitions
                rinv = smp.tile([1, 512], F32, tag="rinv", name="rinv" + sfx)
                nc.vector.reciprocal(rinv[:], acc_ps[0:1, :])
                if debug:
                    nc.sync.dma_start(dbg["dbg_rinv"][:], rinv[:])
                rb_ps = bcp.tile([128, 512], F32, tag="bc", name="rb_ps" + sfx)
                nc.tensor.matmul(rb_ps[:], ones_q[:, :], rinv[:],
                                 start=True, stop=True)
                rb_sb = smp.tile([128, 512], F32, tag="rb_sb",
                                 name="rb_sb" + sfx)
                nc.scalar.copy(rb_sb[:], rb_ps[:])

                # ---------- phase 4: PV -> head^T ----------
                hT = pp.tile([128, 8, TOK], BF, tag="hT", name="hT" + sfx)
                for m in range(8):
                    pv = psp.tile([128, 512], F32, tag="ps",
                                  name=f"pv{m}{sfx}")
                    for p in range(NP):
                        lo, w = _width(p)
                        nc.tensor.matmul(
                            pv[:, lo:512], qn_all[:, p, 128 * m:128 * (m + 1)],
                            ET[:, p, lo:512],
                            start=(p == 0), stop=(p == NP - 1))
                    nc.vector.tensor_tensor(hT[:, m, :], pv[:], rb_sb[:],
                                            op=ALU.mult)
                    nc.vector.tensor_tensor(hT[:, m, :], hT[:, m, :],
                                            qT_my[:, m, :], op=ALU.add)

                # ---------- phase 5: out-proj^T + LN stats ----------
                hT2 = pp.tile([128, 8, TOK], BF, tag="hT2", name="hT2" + sfx)
                ln_sum = acc_ps[32:33, :]
                ln_sq = acc_ps[64:65, :]
                sq_tiles = {}

                def emit_oproj(mo):
                    hp = psp.tile([128, 512], F32, tag="ps",
                                  name=f"hp{mo}{sfx}")
                    for kd in range(8):
                        nc.tensor.matmul(
                            hp[:], weff_bf[:, kd, 128 * mo:128 * (mo + 1)],
                            hT[:, kd, :], start=(kd == 0), stop=False)
                    # bias add on PE: hp += bias_chunk (x) ones_row
                    nc.tensor.matmul(
                        hp[:], bias_sb[:, 128 * mo:128 * (mo + 1)],
                        ones_row[:], start=False, stop=True)
                    nc.scalar.copy(hT2[:, mo, :], hp[:])
                    sq = sqp.tile([128, 512], BF, tag="sq", name=f"sq{mo}{sfx}")
                    sq_tiles[mo] = sq
                    nc.scalar.activation(sq[:], hp[:], AF.Square)

                def emit_lnsum(mo):
                    nc.tensor.matmul(ln_sum, ones_k[:, :], hT2[:, mo, :],
                                     start=(mo == 0), stop=(mo == 7))
                    nc.tensor.matmul(ln_sq, ones_k[:, :], sq_tiles[mo][:],
                                     start=(mo == 0), stop=(mo == 7))

                for mo in range(8):
                    emit_oproj(mo)
                    if mo > 0:
                        emit_lnsum(mo - 1)
                emit_lnsum(7)

                # LN scalars on [1,512].  LayerNorm is folded into FFN1:
                #   f1 = rstd * (wi^T @ hT2 - wsum (x) mean),  wsum = wi.sum(0)
                # so FFN1's matmuls start straight from hT2 (no barrier).
                stat = smp.tile([1, 512 * 4], F32, tag="stat",
                                name="stat" + sfx)
                mean, msq, var, rstd = (stat[:, 512 * k:512 * (k + 1)]
                                        for k in range(4))
                nc.vector.tensor_scalar(mean, ln_sum, 1.0 / D, None,
                                        op0=ALU.mult)
                nc.vector.tensor_scalar(msq, ln_sq, 1.0 / D, None,
                                        op0=ALU.mult)
                nc.vector.tensor_tensor(var, mean, mean, op=ALU.mult)
                nc.vector.tensor_tensor(var, msq, var, op=ALU.subtract)
                nc.scalar.activation(var, var, AF.Sqrt, bias=eps_ap[:, 0:1])
                nc.vector.reciprocal(rstd, var)
                nmean = smp.tile([1, 512], BF, tag="nmean", name="nmean" + sfx)
                nc.vector.tensor_scalar(nmean[:], mean, -1.0, None,
                                        op0=ALU.mult)
                rstd_ps = bcp.tile([128, 512], F32, tag="bc",
                                   name="rstd_ps" + sfx)
                nc.tensor.matmul(rstd_ps[:], ones_q[:, :], rstd,
                                 start=True, stop=True)
                rstd_b = smp.tile([128, 512], F32, tag="rstd_b",
                                  name="rstd_b" + sfx)
                nc.scalar.copy(rstd_b[:], rstd_ps[:])

                # ---------- phase 6: FFN (transposed, LN fused into FFN1) ----
                saT = pp.tile([128, 8, TOK], BF, tag="saT", name="saT" + sfx)
                for m in range(8):
                    f1 = psp.tile([128, 512], F32, tag="ps",
                                  name=f"f1_{m}{sfx}")
                    for mo in range(8):
                        nc.tensor.matmul(
                            f1[:], wi_ap(mo, 128 * m, 128 * (m + 1)),
                            hT2[:, mo, :], start=(mo == 0), stop=False)
                    nc.tensor.matmul(
                        f1[:], wsum_sb[:, 128 * m:128 * (m + 1)], nmean[:],
                        start=False, stop=True)
                    f1s = sqp.tile([128, 512], F32, tag="f1s",
                                   name=f"f1s{m}{sfx}")
                    nc.vector.tensor_tensor(f1s[:], f1[:], rstd_b[:],
                                            op=ALU.mult)
                    nc.scalar.activation(saT[:, m, :], f1s[:], AF.Silu)
                for mo in range(8):
                    f2 = psp.tile([128, 512], F32, tag="ps",
                                  name=f"f2_{mo}{sfx}")
                    for m in range(8):
                        nc.tensor.matmul(
                            f2[:], wi_ap(m, 128 * mo, 128 * (mo + 1)),
                            saT[:, m, :], start=(m == 0), stop=(m == 7))
                    ob = outp.tile([128, 512], F32, tag="outb",
                                   name=f"ob{mo}{sfx}")
                    nc.scalar.copy(ob[:], f2[:])
                    nc.sync.dma_start(
                        out_d[128 * mo:128 * (mo + 1), :], ob[:])

                if debug:
                    nc.sync.dma_start(dbg["dbg_qn_my"][:], qn_my[:])
                    nc.sync.dma_start(dbg["dbg_qT_my"][:], qT_my[:])
                    nc.sync.dma_start(dbg["dbg_qn_all"][:], qn_all[:])
                    nc.sync.dma_start(dbg["dbg_ET"][:], ET[:])
                    nc.sync.dma_start(dbg["dbg_hT"][:], hT[:])
                    nc.sync.dma_start(dbg["dbg_hT2"][:], hT2[:])

    nc.compile()
    return nc


def _get_nc(debug=False, reps=1, sim_cc_as_dma=False):
    key = ("nc", debug, reps, sim_cc_as_dma)
    if key not in _CACHE:
        _CACHE[key] = _build(debug, reps, sim_cc_as_dma)
    return _CACHE[key]


def _granules(r):
    return [r + 4 * i for i in range(4)]


def make_in_maps(x, mask, wi, out_kernel, out_bias):
    """Host-side sharding: build the 8 per-core input dicts."""
    import ml_dtypes

    BFh = ml_dtypes.bfloat16
    x = np.asarray(x, dtype=np.float32)
    wi_bf = np.ascontiguousarray(np.asarray(wi, np.float32).astype(BFh))
    wsum = np.asarray(wi, np.float32).astype(BFh).astype(np.float32) \
        .sum(axis=0).reshape(1, D).astype(BFh)
    weff = np.asarray(out_kernel, np.float32).reshape(H, D, D).sum(axis=0)
    weff_bf = np.ascontiguousarray(weff.astype(BFh))
    bias = np.ascontiguousarray(
        np.asarray(out_bias, np.float32).reshape(1, D).astype(BFh))
    ident = np.eye(128, dtype=BFh)
    mask = np.asarray(mask).astype(bool)
    amadd = np.where(mask, np.float32(0.0), np.float32(MASK_NEG))

    in_maps = []
    for c in range(NCORES):
        b, r = divmod(c, 4)
        gs = _granules(r)
        rows = np.concatenate([np.r_[128 * g:128 * (g + 1)] for g in gs])
        xt = np.ascontiguousarray(x[b, rows, :].T.astype(BFh))  # [D, TOK]
        am = np.empty((NP, 128, 128), dtype=BFh)
        for p in range(NP):
            h = POS_H[p]           # k granule height at position p
            i = p % 4              # masked q slot
            gq = r + 4 * i         # this core's q granule in slot i
            am[p] = amadd[128 * gq:128 * (gq + 1),
                          128 * h:128 * (h + 1)].T.astype(BFh)
        in_maps.append({
            "xt": xt, "wi": wi_bf, "weff": weff_bf, "bias": bias,
            "wsum": wsum, "ident": ident, "amask": am,
        })
    return in_maps


def check_mask_causal(mask):
    """The compiled program skips k granules of height >= 4*(slot+1); that is
    only sound if those blocks are fully masked for every q row that might
    occupy the slot, i.e. for the causal (tril) mask or stricter."""
    mask = np.asarray(mask).astype(bool)
    for i in range(4):
        for r in range(4):
            gq = r + 4 * i
            if mask[128 * gq:128 * (gq + 1), 128 * 4 * (i + 1):].any():
                return False
    return True


def assemble_output(results):
    out = np.empty((B, S, D), dtype=np.float32)
    for c in range(NCORES):
        b, r = divmod(c, 4)
        res = results[c]["out"]                      # [D, TOK]
        for i, g in enumerate(_granules(r)):
            out[b, 128 * g:128 * (g + 1), :] = res[:, 128 * i:128 * (i + 1)].T
    return out


def kernel(x, mask, wi, out_kernel, out_bias, n_heads):
    from concourse.bass_utils import run_bass_kernel_spmd

    assert int(np.asarray(n_heads)) == H
    assert check_mask_causal(mask), "compiled program requires a causal mask"
    nc = _get_nc()
    in_maps = make_in_maps(x, mask, wi, out_kernel, out_bias)
    res = run_bass_kernel_spmd(nc, in_maps, core_ids=list(range(NCORES)))
    return assemble_output(res.results)


if __name__ == "__main__":
    sys.path.insert(0, "/root/problem")
    import reference

    inputs = {k: np.asarray(v) for k, v in reference.setup_inputs().items()}
    exp = np.asarray(reference.reference(**reference.setup_inputs()))
    act = kernel(**inputs)
    err = np.linalg.norm(act - exp) / np.linalg.norm(exp)
    print("Relative error:", err)
